# revision 8
# baseline (speedup 1.0000x reference)
"""Multi-head attention block (QKV linear -> softmax attention -> proj linear)
for Trainium2, SPMD over 8 NeuronCores.

Sharding: 8 shards = batch (4) x head-group (2 groups of 6 heads).
Each core computes, for its (b, g):
    qkv   = x[b] @ Wqkv[:, cols(g)]            (bf16 matmul, fp32 accum)
    S^T_h = K_h Q_h^T   per head               (keys on partitions)
    P^T_h = exp(SCALE * S^T_h)                 (ACT engine, bf16 out)
    out_h = (P_h @ [V_h | 1]) -> normalize rows by the ones-column sum
    y_g   = concat_h(out_h) @ Wproj[rows(g), :]    (partial, fp32 out)
Host sums the two head-group partials per batch and adds bproj.

Shapes hardcoded: x [4, 2048, 768], Wqkv [768, 2304], Wproj [768, 768].
"""

import os
from contextlib import ExitStack

import numpy as np
import ml_dtypes

import concourse.bass as bass
import concourse.mybir as mybir
import concourse.tile as tile
from concourse import bacc
from concourse.bass_utils import run_bass_kernel_spmd
from concourse.masks import make_identity

B, N, C = 4, 2048, 768
H, D = 12, 64          # total heads, head dim
G = 2                  # head groups (tensor-parallel axis)
HL = H // G            # heads per core = 6
SCALE = D ** -0.5
P = 128
CB = C // P            # 6 contraction blocks
NT = N // P            # 16 row tiles
EG = HL * D            # 384 = per-group width of Q / K / V
NCORES = 8

f32 = mybir.dt.float32
bf16 = mybir.dt.bfloat16

# knobs
PT_BUFS = int(os.environ.get("KRN_PT_BUFS", "25"))
SCORE_CHUNK = int(os.environ.get("KRN_SCORE_CHUNK", "1024"))
SCORE_BUFS = int(os.environ.get("KRN_SCORE_BUFS", "3"))
PV_SPLIT = int(os.environ.get("KRN_PV_SPLIT", "2"))  # split-K halves for PV


def _build_program():
    nc = bacc.Bacc("TRN2", target_bir_lowering=False, debug=False)

    xT = nc.dram_tensor("xT", [C, N], bf16, kind="ExternalInput")           # x[b].T
    wqkv = nc.dram_tensor("wqkv", [C, 3 * EG], bf16, kind="ExternalInput")  # [Qg|Kg|Vg]
    wproj = nc.dram_tensor("wproj", [EG, C], bf16, kind="ExternalInput")    # group rows
    y = nc.dram_tensor("y", [N, C], f32, kind="ExternalOutput")             # partial out

    with tile.TileContext(nc) as tc, ExitStack() as ctx:
        persist = ctx.enter_context(tc.tile_pool(name="persist", bufs=1))
        ptpool = ctx.enter_context(tc.tile_pool(name="ptpool", bufs=PT_BUFS))
        rpool = ctx.enter_context(tc.tile_pool(name="rpool", bufs=8))
        ps_score = ctx.enter_context(
            tc.tile_pool(name="ps_score", bufs=SCORE_BUFS, space="PSUM")
        )
        ps_small = ctx.enter_context(tc.tile_pool(name="ps_small", bufs=2, space="PSUM"))

        identity = persist.tile([P, P], bf16, tag="identity")
        make_identity(nc, identity)

        # ---- loads ----
        # x^T lives in six "pt"-tagged tiles: their slots are recycled for
        # P^T once QKV is done, giving attention more probability buffers.
        xts = []
        for cb in range(CB):
            xt_cb = ptpool.tile([P, N], bf16, tag="pt", name=f"xt{cb}")
            nc.sync.dma_start(xt_cb[:], xT[cb * P : (cb + 1) * P, :])
            xts.append(xt_cb)
        wq_sb = persist.tile([P, CB, 3 * EG], bf16, tag="wq")
        nc.sync.dma_start(wq_sb[:], wqkv[:].rearrange("(cb p) e -> p cb e", p=P))
        wp_sb = persist.tile([P, EG // P, C], bf16, tag="wp")
        nc.sync.dma_start(wp_sb[:], wproj[:].rearrange("(cb p) c -> p cb c", p=P))

        # ---- QKV ----
        # qkT_sb blocks 0..2 = Q^T (row h*64+d), blocks 3..5 = K^T
        qkT_sb = persist.tile([P, 2 * EG // P, N], bf16, tag="qkT")
        # V' [m, 6*65]: per head 64 V columns + a ones column (row-sum trick)
        vp_sb = persist.tile([P, NT, HL * (D + 1)], bf16, tag="vp")
        vp4 = vp_sb.rearrange("p m (h c) -> p m h c", c=D + 1)
        nc.vector.memset(vp4[:, :, :, D : D + 1], 1.0)

        # Q^T / K^T: out[e128, n512] = Wg[:, eb].T @ xT; emit K blocks early
        # so head-0 scores can start before all of QKV is done.
        for eb in (3, 0, 4, 1, 5, 2):
            for nch in range(N // 512):
                qpsum = ps_small.tile([P, 512], f32, tag="sm")
                for cb in range(CB):
                    nc.tensor.matmul(
                        qpsum,
                        wq_sb[:, cb, eb * P : (eb + 1) * P],
                        xts[cb][:, nch * 512 : (nch + 1) * 512],
                        start=(cb == 0),
                        stop=(cb == CB - 1),
                    )
                nc.vector.tensor_copy(
                    qkT_sb[:, eb, nch * 512 : (nch + 1) * 512], qpsum
                )

        # V: out[m128, 384] = xT[:, mt].T @ Wv; scatter into V' (65-stride)
        for mt in range(NT):
            vpsum = ps_small.tile([P, 512], f32, tag="sm")
            for cb in range(CB):
                nc.tensor.matmul(
                    vpsum[:, :EG],
                    xts[cb][:, mt * P : (mt + 1) * P],
                    wq_sb[:, cb, 2 * EG : 3 * EG],
                    start=(cb == 0),
                    stop=(cb == CB - 1),
                )
            nc.vector.tensor_copy(
                vp4[:, mt, :, :D],
                vpsum[:, :EG].rearrange("p (h d) -> p h d", d=D),
            )

        # ---- attention ----
        og_sb = persist.tile([P, NT, EG], bf16, tag="og")  # heads out [n, ch]
        pvstage = persist.tile([P, NT, D + 1], f32, tag="pvst")  # PV partials
        n_chunks = N // SCORE_CHUNK
        for h in range(HL):
            prow = (h % 2) * D
            qblk = h // 2
            kblk = 3 + h // 2
            pts = []
            for mt in range(NT):
                pt = ptpool.tile([P, N], bf16, tag="pt")
                pts.append(pt)
                for nch in range(n_chunks):
                    spsum = ps_score.tile([P, SCORE_CHUNK], f32)
                    for sub in range(SCORE_CHUNK // 512):
                        off = nch * SCORE_CHUNK + sub * 512
                        nc.tensor.matmul(
                            spsum[:, sub * 512 : (sub + 1) * 512],
                            qkT_sb[prow : prow + D, kblk, mt * P : (mt + 1) * P],
                            qkT_sb[prow : prow + D, qblk, off : off + 512],
                            start=True,
                            stop=True,
                        )
                    nc.scalar.activation(
                        pt[:, nch * SCORE_CHUNK : (nch + 1) * SCORE_CHUNK],
                        spsum,
                        mybir.ActivationFunctionType.Exp,
                        scale=SCALE,
                    )
            # split-K PV: each half releases its P^T tiles early so the
            # next head's scores/exp are never slot-starved.
            mt_per = NT // PV_SPLIT
            for half in range(PV_SPLIT):
                m0 = half * mt_per
                for nt in range(NT):
                    pvpsum = ps_small.tile([P, 512], f32, tag="sm")
                    for mt in range(m0, m0 + mt_per):
                        nc.tensor.matmul(
                            pvpsum[:, : D + 1],
                            pts[mt][:, nt * P : (nt + 1) * P],
                            vp_sb[:, mt, h * (D + 1) : (h + 1) * (D + 1)],
                            start=(mt == m0),
                            stop=(mt == m0 + mt_per - 1),
                        )
                    if PV_SPLIT == 1:
                        r = rpool.tile([P, 1], f32)
                        nc.vector.reciprocal(r, pvpsum[:, D : D + 1])
                        nc.vector.tensor_scalar(
                            og_sb[:, nt, h * D : (h + 1) * D],
                            pvpsum[:, :D],
                            r,
                            None,
                            mybir.AluOpType.mult,
                        )
                    elif half == 0:
                        nc.vector.tensor_copy(pvstage[:, nt], pvpsum[:, : D + 1])
                    elif half < PV_SPLIT - 1:
                        nc.vector.tensor_tensor(
                            pvstage[:, nt],
                            pvpsum[:, : D + 1],
                            pvstage[:, nt],
                            mybir.AluOpType.add,
                        )
                    else:
                        sfin = rpool.tile([P, D + 1], f32, tag="sfin")
                        nc.vector.tensor_tensor(
                            sfin,
                            pvpsum[:, : D + 1],
                            pvstage[:, nt],
                            mybir.AluOpType.add,
                        )
                        r = rpool.tile([P, 1], f32)
                        nc.vector.reciprocal(r, sfin[:, D : D + 1])
                        nc.vector.tensor_scalar(
                            og_sb[:, nt, h * D : (h + 1) * D],
                            sfin[:, :D],
                            r,
                            None,
                            mybir.AluOpType.mult,
                        )

        # ---- transpose heads-out to [ch, n] for proj ----
        ogT_sb = persist.tile([P, EG // P, N], bf16, tag="ogT")
        for nt in range(NT):
            for cb in range(EG // P):
                tpsum = ps_small.tile([P, 512], bf16, tag="sm")
                nc.tensor.transpose(
                    tpsum[:, :P], og_sb[:, nt, cb * P : (cb + 1) * P], identity
                )
                nc.vector.tensor_copy(
                    ogT_sb[:, cb, nt * P : (nt + 1) * P], tpsum[:, :P]
                )

        # ---- proj ----
        ypool = ctx.enter_context(tc.tile_pool(name="ypool", bufs=2))
        yv = y[:].rearrange("(nt p) c -> p nt c", p=P)
        for nt in range(NT):
            y_sb = ypool.tile([P, C], f32)
            for half in range(2):
                ppsum = ps_small.tile([P, 512], f32, tag="sm")
                for cb in range(EG // P):
                    nc.tensor.matmul(
                        ppsum[:, :EG],
                        ogT_sb[:, cb, nt * P : (nt + 1) * P],
                        wp_sb[:, cb, half * EG : (half + 1) * EG],
                        start=(cb == 0),
                        stop=(cb == EG // P - 1),
                    )
                nc.vector.tensor_copy(
                    y_sb[:, half * EG : (half + 1) * EG], ppsum[:, :EG]
                )
            nc.sync.dma_start(yv[:, nt], y_sb)

    nc.compile()
    return nc


_PROGRAM = None


def _get_program():
    global _PROGRAM
    if _PROGRAM is None:
        _PROGRAM = _build_program()
    return _PROGRAM


def _shard_inputs(x, Wqkv, Wproj):
    bf = ml_dtypes.bfloat16
    in_maps = []
    for core in range(NCORES):
        b, g = core // G, core % G
        xT = np.ascontiguousarray(x[b].T).astype(bf)
        wg = np.concatenate(
            [
                Wqkv[:, g * EG : (g + 1) * EG],
                Wqkv[:, C + g * EG : C + (g + 1) * EG],
                Wqkv[:, 2 * C + g * EG : 2 * C + (g + 1) * EG],
            ],
            axis=1,
        ).astype(bf)
        wp = np.ascontiguousarray(Wproj[g * EG : (g + 1) * EG, :]).astype(bf)
        in_maps.append({"xT": xT, "wqkv": wg, "wproj": wp})
    return in_maps


def _run(x, Wqkv, Wproj, bproj, trace=False):
    nc = _get_program()
    in_maps = _shard_inputs(x, Wqkv, Wproj)
    res = run_bass_kernel_spmd(nc, in_maps, list(range(NCORES)), trace=trace)
    out = np.empty((B, N, C), np.float32)
    for b in range(B):
        out[b] = res.results[b * G]["y"] + res.results[b * G + 1]["y"] + bproj
    return out, res


def kernel(x, Wqkv, Wproj, bproj):
    x = np.asarray(x, np.float32)
    Wqkv = np.asarray(Wqkv, np.float32)
    Wproj = np.asarray(Wproj, np.float32)
    bproj = np.asarray(bproj, np.float32)
    out, _ = _run(x, Wqkv, Wproj, bproj)
    return out


# revision 10
# speedup vs baseline: 1.0227x; 1.0227x over previous
"""Multi-head attention block (QKV linear -> softmax attention -> proj linear)
for Trainium2, SPMD over 8 NeuronCores.

Sharding: 8 shards = batch (4) x head-group (2 groups of 6 heads).
Each core computes, for its (b, g):
    qkv   = x[b] @ Wqkv[:, cols(g)]            (bf16 matmul, fp32 accum)
    S^T_h = K_h Q_h^T   per head               (keys on partitions)
    P^T_h = exp(SCALE * S^T_h)                 (ACT engine, bf16 out)
    out_h = (P_h @ [V_h | 1]) -> normalize rows by the ones-column sum
    y_g   = concat_h(out_h) @ Wproj[rows(g), :]    (partial, fp32 out)
Host sums the two head-group partials per batch and adds bproj.

Shapes hardcoded: x [4, 2048, 768], Wqkv [768, 2304], Wproj [768, 768].
"""

import os
from contextlib import ExitStack

import numpy as np
import ml_dtypes

import concourse.bass as bass
import concourse.mybir as mybir
import concourse.tile as tile
from concourse import bacc
from concourse.bass_utils import run_bass_kernel_spmd
from concourse.masks import make_identity

B, N, C = 4, 2048, 768
H, D = 12, 64          # total heads, head dim
G = 2                  # head groups (tensor-parallel axis)
HL = H // G            # heads per core = 6
SCALE = D ** -0.5
P = 128
CB = C // P            # 6 contraction blocks
NT = N // P            # 16 row tiles
EG = HL * D            # 384 = per-group width of Q / K / V
NCORES = 8

f32 = mybir.dt.float32
bf16 = mybir.dt.bfloat16

# knobs
PT_BUFS = int(os.environ.get("KRN_PT_BUFS", "25"))
PV_SPLIT = int(os.environ.get("KRN_PV_SPLIT", "1"))  # split-K pieces for PV


def _build_program():
    nc = bacc.Bacc("TRN2", target_bir_lowering=False, debug=False)

    xT = nc.dram_tensor("xT", [C, N], bf16, kind="ExternalInput")           # x[b].T
    wqkv = nc.dram_tensor("wqkv", [C, 3 * EG], bf16, kind="ExternalInput")  # [Qg|Kg|Vg]
    wproj = nc.dram_tensor("wproj", [EG, C], bf16, kind="ExternalInput")    # group rows
    y = nc.dram_tensor("y", [N, C], f32, kind="ExternalOutput")             # partial out

    with tile.TileContext(nc) as tc, ExitStack() as ctx:
        persist = ctx.enter_context(tc.tile_pool(name="persist", bufs=1))
        ptpool = ctx.enter_context(tc.tile_pool(name="ptpool", bufs=PT_BUFS))
        rpool = ctx.enter_context(tc.tile_pool(name="rpool", bufs=8))
        ypool = ctx.enter_context(tc.tile_pool(name="ypool", bufs=2))
        ps_a = ctx.enter_context(tc.tile_pool(name="ps_a", bufs=1, space="PSUM"))
        ps_b = ctx.enter_context(tc.tile_pool(name="ps_b", bufs=1, space="PSUM"))
        ps_small = ctx.enter_context(tc.tile_pool(name="ps_small", bufs=2, space="PSUM"))

        identity = persist.tile([P, P], bf16, tag="identity")
        make_identity(nc, identity)

        # ---- loads ----
        # x^T lives in six "pt"-tagged tiles: their slots are recycled for
        # P^T once the V phase is done.
        xts = []
        for cb in range(CB):
            xt_cb = ptpool.tile([P, N], bf16, tag="pt", name=f"xt{cb}")
            nc.sync.dma_start(xt_cb[:], xT[cb * P : (cb + 1) * P, :])
            xts.append(xt_cb)
        wq_sb = persist.tile([P, CB, 3 * EG], bf16, tag="wq")
        nc.sync.dma_start(wq_sb[:], wqkv[:].rearrange("(cb p) e -> p cb e", p=P))
        wp_sb = persist.tile([P, EG // P, C], bf16, tag="wp")
        nc.sync.dma_start(wp_sb[:], wproj[:].rearrange("(cb p) c -> p cb c", p=P))

        # persistent attention tiles
        qkT_sb = persist.tile([P, 2 * EG // P, N], bf16, tag="qkT")
        vp_sb = persist.tile([P, NT, HL * (D + 1)], bf16, tag="vp")
        vp4 = vp_sb.rearrange("p m (h c) -> p m h c", c=D + 1)
        nc.vector.memset(vp4[:, :, :, D : D + 1], 1.0)
        og_sb = persist.tile([P, NT, EG], bf16, tag="og")   # heads out [n, ch]
        pvstage = persist.tile([P, NT, D + 1], f32, tag="pvst")

        def emit_qk(eb):
            # Q^T / K^T: out[e128, n512] = Wg[:, eb].T @ x^T
            for nch in range(N // 512):
                qpsum = ps_small.tile([P, 512], f32, tag="sm")
                for cb in range(CB):
                    nc.tensor.matmul(
                        qpsum,
                        wq_sb[:, cb, eb * P : (eb + 1) * P],
                        xts[cb][:, nch * 512 : (nch + 1) * 512],
                        start=(cb == 0),
                        stop=(cb == CB - 1),
                    )
                nc.vector.tensor_copy(
                    qkT_sb[:, eb, nch * 512 : (nch + 1) * 512], qpsum
                )

        def emit_v():
            # V: out[m128, 384] = x^T[:, mt].T @ Wv; scatter into V' (65-stride)
            for mt in range(NT):
                vpsum = ps_small.tile([P, 512], f32, tag="sm")
                for cb in range(CB):
                    nc.tensor.matmul(
                        vpsum[:, :EG],
                        xts[cb][:, mt * P : (mt + 1) * P],
                        wq_sb[:, cb, 2 * EG : 3 * EG],
                        start=(cb == 0),
                        stop=(cb == CB - 1),
                    )
                nc.vector.tensor_copy(
                    vp4[:, mt, :, :D],
                    vpsum[:, :EG].rearrange("p (h d) -> p h d", d=D),
                )

        def emit_scores(h):
            """Scores + exp for one head; returns the P^T tiles.

            Asymmetric psum ping-pong: even m-tiles use the 4-bank buffer
            (one exp over 2048 columns), odd m-tiles two exps over 1024 from
            the 2-bank buffer -- bigger exps amortize ACT's fixed per-
            instruction overhead while scores + PV + evict psums still fit
            in 8 banks."""
            prow = (h % 2) * D
            qblk = h // 2
            kblk = 3 + h // 2
            pts = []
            for mt in range(NT):
                pt = ptpool.tile([P, N], bf16, tag="pt")
                pts.append(pt)
                lhsT = qkT_sb[prow : prow + D, kblk, mt * P : (mt + 1) * P]
                if mt % 2 == 0:
                    spsum = ps_a.tile([P, 2048], f32)
                    for sub in range(4):
                        nc.tensor.matmul(
                            spsum[:, sub * 512 : (sub + 1) * 512],
                            lhsT,
                            qkT_sb[prow : prow + D, qblk, sub * 512 : (sub + 1) * 512],
                            start=True,
                            stop=True,
                        )
                    nc.scalar.activation(
                        pt[:], spsum, mybir.ActivationFunctionType.Exp, scale=SCALE
                    )
                else:
                    for nch in range(2):
                        spsum = ps_b.tile([P, 1024], f32)
                        for sub in range(2):
                            off = nch * 1024 + sub * 512
                            nc.tensor.matmul(
                                spsum[:, sub * 512 : (sub + 1) * 512],
                                lhsT,
                                qkT_sb[prow : prow + D, qblk, off : off + 512],
                                start=True,
                                stop=True,
                            )
                        nc.scalar.activation(
                            pt[:, nch * 1024 : (nch + 1) * 1024],
                            spsum,
                            mybir.ActivationFunctionType.Exp,
                            scale=SCALE,
                        )
            return pts

        def emit_pv(h, pts):
            mt_per = NT // PV_SPLIT
            for half in range(PV_SPLIT):
                m0 = half * mt_per
                for nt in range(NT):
                    pvpsum = ps_small.tile([P, 512], f32, tag="sm")
                    for mt in range(m0, m0 + mt_per):
                        nc.tensor.matmul(
                            pvpsum[:, : D + 1],
                            pts[mt][:, nt * P : (nt + 1) * P],
                            vp_sb[:, mt, h * (D + 1) : (h + 1) * (D + 1)],
                            start=(mt == m0),
                            stop=(mt == m0 + mt_per - 1),
                        )
                    if PV_SPLIT == 1:
                        r = rpool.tile([P, 1], f32)
                        nc.vector.reciprocal(r, pvpsum[:, D : D + 1])
                        nc.vector.tensor_scalar(
                            og_sb[:, nt, h * D : (h + 1) * D],
                            pvpsum[:, :D],
                            r,
                            None,
                            mybir.AluOpType.mult,
                        )
                    elif half == 0:
                        nc.vector.tensor_copy(pvstage[:, nt], pvpsum[:, : D + 1])
                    elif half < PV_SPLIT - 1:
                        nc.vector.tensor_tensor(
                            pvstage[:, nt],
                            pvpsum[:, : D + 1],
                            pvstage[:, nt],
                            mybir.AluOpType.add,
                        )
                    else:
                        sfin = rpool.tile([P, D + 1], f32, tag="sfin")
                        nc.vector.tensor_tensor(
                            sfin,
                            pvpsum[:, : D + 1],
                            pvstage[:, nt],
                            mybir.AluOpType.add,
                        )
                        r = rpool.tile([P, 1], f32)
                        nc.vector.reciprocal(r, sfin[:, D : D + 1])
                        nc.vector.tensor_scalar(
                            og_sb[:, nt, h * D : (h + 1) * D],
                            sfin[:, :D],
                            r,
                            None,
                            mybir.AluOpType.mult,
                        )

        # ---- emission schedule ----
        # K/Q blocks for head 0+1 first, then head-0 scores so ACT starts
        # early; V + remaining blocks overlap with head-0 exps.
        emit_qk(3)
        emit_qk(0)
        all_pts = [emit_scores(0)]
        emit_qk(4)
        emit_qk(1)
        emit_qk(5)
        emit_qk(2)
        emit_v()
        for h in range(1, HL):
            all_pts.append(emit_scores(h))
            emit_pv(h - 1, all_pts[h - 1])
        emit_pv(HL - 1, all_pts[HL - 1])

        # ---- transpose heads-out to [ch, n] for proj ----
        ogT_sb = persist.tile([P, EG // P, N], bf16, tag="ogT")
        for nt in range(NT):
            for cb in range(EG // P):
                tpsum = ps_small.tile([P, 512], bf16, tag="sm")
                nc.tensor.transpose(
                    tpsum[:, :P], og_sb[:, nt, cb * P : (cb + 1) * P], identity
                )
                nc.vector.tensor_copy(
                    ogT_sb[:, cb, nt * P : (nt + 1) * P], tpsum[:, :P]
                )

        # ---- proj ----
        yv = y[:].rearrange("(nt p) c -> p nt c", p=P)
        for nt in range(NT):
            y_sb = ypool.tile([P, C], f32)
            for half in range(2):
                ppsum = ps_small.tile([P, 512], f32, tag="sm")
                for cb in range(EG // P):
                    nc.tensor.matmul(
                        ppsum[:, :EG],
                        ogT_sb[:, cb, nt * P : (nt + 1) * P],
                        wp_sb[:, cb, half * EG : (half + 1) * EG],
                        start=(cb == 0),
                        stop=(cb == EG // P - 1),
                    )
                nc.vector.tensor_copy(
                    y_sb[:, half * EG : (half + 1) * EG], ppsum[:, :EG]
                )
            nc.sync.dma_start(yv[:, nt], y_sb)

    nc.compile()
    return nc


_PROGRAM = None


def _get_program():
    global _PROGRAM
    if _PROGRAM is None:
        _PROGRAM = _build_program()
    return _PROGRAM


def _shard_inputs(x, Wqkv, Wproj):
    bf = ml_dtypes.bfloat16
    in_maps = []
    for core in range(NCORES):
        b, g = core // G, core % G
        xT = np.ascontiguousarray(x[b].T).astype(bf)
        wg = np.concatenate(
            [
                Wqkv[:, g * EG : (g + 1) * EG],
                Wqkv[:, C + g * EG : C + (g + 1) * EG],
                Wqkv[:, 2 * C + g * EG : 2 * C + (g + 1) * EG],
            ],
            axis=1,
        ).astype(bf)
        wp = np.ascontiguousarray(Wproj[g * EG : (g + 1) * EG, :]).astype(bf)
        in_maps.append({"xT": xT, "wqkv": wg, "wproj": wp})
    return in_maps


def _run(x, Wqkv, Wproj, bproj, trace=False):
    nc = _get_program()
    in_maps = _shard_inputs(x, Wqkv, Wproj)
    res = run_bass_kernel_spmd(nc, in_maps, list(range(NCORES)), trace=trace)
    out = np.empty((B, N, C), np.float32)
    for b in range(B):
        out[b] = res.results[b * G]["y"] + res.results[b * G + 1]["y"] + bproj
    return out, res


def kernel(x, Wqkv, Wproj, bproj):
    x = np.asarray(x, np.float32)
    Wqkv = np.asarray(Wqkv, np.float32)
    Wproj = np.asarray(Wproj, np.float32)
    bproj = np.asarray(bproj, np.float32)
    out, _ = _run(x, Wqkv, Wproj, bproj)
    return out


# revision 11
# speedup vs baseline: 1.1900x; 1.1636x over previous
"""Multi-head attention block (QKV linear -> softmax attention -> proj linear)
for Trainium2, SPMD over 8 NeuronCores.

Sharding: 8 shards = batch (4) x head-group (2 groups of 6 heads).
Each core computes, for its (b, g):
    qkv   = x[b] @ Wqkv[:, cols(g)]            (bf16 matmul, fp32 accum)
    S^T_h = K_h Q_h^T   per head               (keys on partitions)
    P^T_h = exp(SCALE * S^T_h)                 (ACT engine, bf16 out)
    out_h = (P_h @ [V_h | 1]) -> normalize rows by the ones-column sum
    y_g   = concat_h(out_h) @ Wproj[rows(g), :]    (partial, fp32 out)
Host sums the two head-group partials per batch and adds bproj.

Shapes hardcoded: x [4, 2048, 768], Wqkv [768, 2304], Wproj [768, 768].
"""

import os
from contextlib import ExitStack

import numpy as np
import ml_dtypes

import concourse.bass as bass
import concourse.mybir as mybir
import concourse.tile as tile
from concourse import bacc
from concourse.bass_utils import run_bass_kernel_spmd
from concourse.masks import make_identity

B, N, C = 4, 2048, 768
H, D = 12, 64          # total heads, head dim
G = 2                  # head groups (tensor-parallel axis)
HL = H // G            # heads per core = 6
SCALE = D ** -0.5
P = 128
CB = C // P            # 6 contraction blocks
NT = N // P            # 16 row tiles
EG = HL * D            # 384 = per-group width of Q / K / V
NCORES = 8

f32 = mybir.dt.float32
bf16 = mybir.dt.bfloat16

# knobs
PT_BUFS = int(os.environ.get("KRN_PT_BUFS", "25"))
PV_SPLIT = int(os.environ.get("KRN_PV_SPLIT", "1"))  # split-K pieces for PV


def _build_program():
    nc = bacc.Bacc("TRN2", target_bir_lowering=False, debug=False)

    xT = nc.dram_tensor("xT", [C, N], bf16, kind="ExternalInput")           # x[b].T
    wqkv = nc.dram_tensor("wqkv", [C, 3 * EG], bf16, kind="ExternalInput")  # [Qg|Kg|Vg]
    wproj = nc.dram_tensor("wproj", [EG, C], bf16, kind="ExternalInput")    # group rows
    y = nc.dram_tensor("y", [N, C], f32, kind="ExternalOutput")             # partial out

    with tile.TileContext(nc) as tc, ExitStack() as ctx:
        persist = ctx.enter_context(tc.tile_pool(name="persist", bufs=1))
        ptpool = ctx.enter_context(tc.tile_pool(name="ptpool", bufs=PT_BUFS))
        rpool = ctx.enter_context(tc.tile_pool(name="rpool", bufs=8))
        ypool = ctx.enter_context(tc.tile_pool(name="ypool", bufs=2))
        ps_score = ctx.enter_context(tc.tile_pool(name="ps_score", bufs=3, space="PSUM"))
        ps_small = ctx.enter_context(tc.tile_pool(name="ps_small", bufs=2, space="PSUM"))

        identity = persist.tile([P, P], bf16, tag="identity")
        make_identity(nc, identity)

        # ---- loads ----
        # x^T lives in six "pt"-tagged tiles: their slots are recycled for
        # P^T once the V phase is done.
        xts = []
        for cb in range(CB):
            xt_cb = ptpool.tile([P, N], bf16, tag="pt", name=f"xt{cb}")
            nc.sync.dma_start(xt_cb[:], xT[cb * P : (cb + 1) * P, :])
            xts.append(xt_cb)
        wq_sb = persist.tile([P, CB, 3 * EG], bf16, tag="wq")
        nc.sync.dma_start(wq_sb[:], wqkv[:].rearrange("(cb p) e -> p cb e", p=P))
        wp_sb = persist.tile([P, EG // P, C], bf16, tag="wp")
        nc.sync.dma_start(wp_sb[:], wproj[:].rearrange("(cb p) c -> p cb c", p=P))

        # persistent attention tiles
        qkT_sb = persist.tile([P, 2 * EG // P, N], bf16, tag="qkT")
        vp_sb = persist.tile([P, NT, HL * (D + 1)], bf16, tag="vp")
        vp4 = vp_sb.rearrange("p m (h c) -> p m h c", c=D + 1)
        nc.vector.memset(vp4[:, :, :, D : D + 1], 1.0)
        og_sb = persist.tile([P, NT, EG], bf16, tag="og")   # heads out [n, ch]
        pvstage = persist.tile([P, NT, D + 1], f32, tag="pvst")

        def emit_qk(eb):
            # Q^T / K^T: out[e128, n512] = Wg[:, eb].T @ x^T
            for nch in range(N // 512):
                qpsum = ps_small.tile([P, 512], f32, tag="sm")
                for cb in range(CB):
                    nc.tensor.matmul(
                        qpsum,
                        wq_sb[:, cb, eb * P : (eb + 1) * P],
                        xts[cb][:, nch * 512 : (nch + 1) * 512],
                        start=(cb == 0),
                        stop=(cb == CB - 1),
                    )
                nc.vector.tensor_copy(
                    qkT_sb[:, eb, nch * 512 : (nch + 1) * 512], qpsum
                )

        def emit_v():
            # V: out[m128, 384] = x^T[:, mt].T @ Wv; scatter into V' (65-stride)
            for mt in range(NT):
                vpsum = ps_small.tile([P, 512], f32, tag="sm")
                for cb in range(CB):
                    nc.tensor.matmul(
                        vpsum[:, :EG],
                        xts[cb][:, mt * P : (mt + 1) * P],
                        wq_sb[:, cb, 2 * EG : 3 * EG],
                        start=(cb == 0),
                        stop=(cb == CB - 1),
                    )
                nc.vector.tensor_copy(
                    vp4[:, mt, :, :D],
                    vpsum[:, :EG].rearrange("p (h d) -> p h d", d=D),
                )

        def emit_scores(h, pv_chaser=None):
            """Scores + exp for one head; returns the P^T tiles.

            Asymmetric psum ping-pong: even m-tiles use the 4-bank buffer
            (one exp over 2048 columns), odd m-tiles two exps over 1024 from
            the 2-bank buffer -- bigger exps amortize ACT's fixed per-
            instruction overhead while scores + PV + evict psums still fit
            in 8 banks."""
            prow = (h % 2) * D
            qblk = h // 2
            kblk = 3 + h // 2
            pts = []
            for mt in range(NT):
                pt = ptpool.tile([P, N], bf16, tag="pt")
                pts.append(pt)
                lhsT = qkT_sb[prow : prow + D, kblk, mt * P : (mt + 1) * P]
                for nch in range(2):
                    spsum = ps_score.tile([P, 1024], f32)
                    for sub in range(2):
                        off = nch * 1024 + sub * 512
                        nc.tensor.matmul(
                            spsum[:, sub * 512 : (sub + 1) * 512],
                            lhsT,
                            qkT_sb[prow : prow + D, qblk, off : off + 512],
                            start=True,
                            stop=True,
                        )
                    nc.scalar.activation(
                        pt[:, nch * 1024 : (nch + 1) * 1024],
                        spsum,
                        mybir.ActivationFunctionType.Exp,
                        scale=SCALE,
                    )
                if pv_chaser is not None:
                    pv_chaser(mt)
            return pts

        def emit_pv_group(h, pts, nt):
            pvpsum = ps_small.tile([P, 512], f32, tag="sm")
            for mt in range(NT):
                nc.tensor.matmul(
                    pvpsum[:, : D + 1],
                    pts[mt][:, nt * P : (nt + 1) * P],
                    vp_sb[:, mt, h * (D + 1) : (h + 1) * (D + 1)],
                    start=(mt == 0),
                    stop=(mt == NT - 1),
                )
            r = rpool.tile([P, 1], f32)
            nc.vector.reciprocal(r, pvpsum[:, D : D + 1])
            nc.vector.tensor_scalar(
                og_sb[:, nt, h * D : (h + 1) * D],
                pvpsum[:, :D],
                r,
                None,
                mybir.AluOpType.mult,
            )

        def emit_pv(h, pts):
            mt_per = NT // PV_SPLIT
            for half in range(PV_SPLIT):
                m0 = half * mt_per
                for nt in range(NT):
                    pvpsum = ps_small.tile([P, 512], f32, tag="sm")
                    for mt in range(m0, m0 + mt_per):
                        nc.tensor.matmul(
                            pvpsum[:, : D + 1],
                            pts[mt][:, nt * P : (nt + 1) * P],
                            vp_sb[:, mt, h * (D + 1) : (h + 1) * (D + 1)],
                            start=(mt == m0),
                            stop=(mt == m0 + mt_per - 1),
                        )
                    if PV_SPLIT == 1:
                        r = rpool.tile([P, 1], f32)
                        nc.vector.reciprocal(r, pvpsum[:, D : D + 1])
                        nc.vector.tensor_scalar(
                            og_sb[:, nt, h * D : (h + 1) * D],
                            pvpsum[:, :D],
                            r,
                            None,
                            mybir.AluOpType.mult,
                        )
                    elif half == 0:
                        nc.vector.tensor_copy(pvstage[:, nt], pvpsum[:, : D + 1])
                    elif half < PV_SPLIT - 1:
                        nc.vector.tensor_tensor(
                            pvstage[:, nt],
                            pvpsum[:, : D + 1],
                            pvstage[:, nt],
                            mybir.AluOpType.add,
                        )
                    else:
                        sfin = rpool.tile([P, D + 1], f32, tag="sfin")
                        nc.vector.tensor_tensor(
                            sfin,
                            pvpsum[:, : D + 1],
                            pvstage[:, nt],
                            mybir.AluOpType.add,
                        )
                        r = rpool.tile([P, 1], f32)
                        nc.vector.reciprocal(r, sfin[:, D : D + 1])
                        nc.vector.tensor_scalar(
                            og_sb[:, nt, h * D : (h + 1) * D],
                            sfin[:, :D],
                            r,
                            None,
                            mybir.AluOpType.mult,
                        )

        # ---- emission schedule ----
        # K/Q blocks for head 0+1 first, then head-0 scores so ACT starts
        # early; V + remaining blocks overlap with head-0 exps.
        emit_qk(3)
        emit_qk(0)
        all_pts = [emit_scores(0)]
        emit_qk(4)
        emit_qk(1)
        emit_qk(5)
        emit_qk(2)
        emit_v()
        for h in range(1, HL):
            hh = h

            def chaser(nt, hh=hh):
                emit_pv_group(hh - 1, all_pts[hh - 1], nt)

            all_pts.append(emit_scores(h, pv_chaser=chaser))
        emit_pv(HL - 1, all_pts[HL - 1])

        # ---- transpose heads-out to [ch, n] for proj ----
        ogT_sb = persist.tile([P, EG // P, N], bf16, tag="ogT")
        for nt in range(NT):
            for cb in range(EG // P):
                tpsum = ps_small.tile([P, 512], bf16, tag="sm")
                nc.tensor.transpose(
                    tpsum[:, :P], og_sb[:, nt, cb * P : (cb + 1) * P], identity
                )
                nc.vector.tensor_copy(
                    ogT_sb[:, cb, nt * P : (nt + 1) * P], tpsum[:, :P]
                )

        # ---- proj ----
        yv = y[:].rearrange("(nt p) c -> p nt c", p=P)
        for nt in range(NT):
            y_sb = ypool.tile([P, C], f32)
            for half in range(2):
                ppsum = ps_small.tile([P, 512], f32, tag="sm")
                for cb in range(EG // P):
                    nc.tensor.matmul(
                        ppsum[:, :EG],
                        ogT_sb[:, cb, nt * P : (nt + 1) * P],
                        wp_sb[:, cb, half * EG : (half + 1) * EG],
                        start=(cb == 0),
                        stop=(cb == EG // P - 1),
                    )
                nc.vector.tensor_copy(
                    y_sb[:, half * EG : (half + 1) * EG], ppsum[:, :EG]
                )
            nc.sync.dma_start(yv[:, nt], y_sb)

    nc.compile()
    return nc


_PROGRAM = None


def _get_program():
    global _PROGRAM
    if _PROGRAM is None:
        _PROGRAM = _build_program()
    return _PROGRAM


def _shard_inputs(x, Wqkv, Wproj):
    bf = ml_dtypes.bfloat16
    in_maps = []
    for core in range(NCORES):
        b, g = core // G, core % G
        xT = np.ascontiguousarray(x[b].T).astype(bf)
        wg = np.concatenate(
            [
                Wqkv[:, g * EG : (g + 1) * EG],
                Wqkv[:, C + g * EG : C + (g + 1) * EG],
                Wqkv[:, 2 * C + g * EG : 2 * C + (g + 1) * EG],
            ],
            axis=1,
        ).astype(bf)
        wp = np.ascontiguousarray(Wproj[g * EG : (g + 1) * EG, :]).astype(bf)
        in_maps.append({"xT": xT, "wqkv": wg, "wproj": wp})
    return in_maps


def _run(x, Wqkv, Wproj, bproj, trace=False):
    nc = _get_program()
    in_maps = _shard_inputs(x, Wqkv, Wproj)
    res = run_bass_kernel_spmd(nc, in_maps, list(range(NCORES)), trace=trace)
    out = np.empty((B, N, C), np.float32)
    for b in range(B):
        out[b] = res.results[b * G]["y"] + res.results[b * G + 1]["y"] + bproj
    return out, res


def kernel(x, Wqkv, Wproj, bproj):
    x = np.asarray(x, np.float32)
    Wqkv = np.asarray(Wqkv, np.float32)
    Wproj = np.asarray(Wproj, np.float32)
    bproj = np.asarray(bproj, np.float32)
    out, _ = _run(x, Wqkv, Wproj, bproj)
    return out
